# revision 10
# baseline (speedup 1.0000x reference)
"""Trainium2 Bass kernel for the contrastive loss problem.

Sharding: core c handles sentence-loss for secrets [4c, 4c+4) (upper-triangle
tiles of the BxB distance matrices, x2-minus-diagonal trick) and secret-loss
for batch columns [128c, 128c+128). Per-core scalar partials are summed on the
host (equivalent to the all-reduce of the scalar losses).
"""

import sys

sys.path.insert(0, "/opt/trn_rl_repo")

import numpy as np
import ml_dtypes

import concourse.bacc as bacc
import concourse.tile as tile
from concourse import mybir
from concourse.bass_utils import run_bass_kernel_spmd

N, B, D = 32, 1024, 1024
NCORES = 8
SECPC = N // NCORES  # 4 secrets per core (sentence term)
BSH = B // NCORES  # 128 batch columns per core (secret term)
EPS = 1e-12
MARGIN = 1.0
ALPHA = 0.5
RSQRT2 = 0.7071067811865476  # Square(x * 1/sqrt(2)) == x^2 / 2

f32 = mybir.dt.float32
bf16 = mybir.dt.bfloat16
fp16 = mybir.dt.float16
Alu = mybir.AluOpType
Act = mybir.ActivationFunctionType
AxX = mybir.AxisListType.X


def _segs(mi):
    """Column segments (start, width<=512) covering [128*mi, 1024)."""
    out = []
    s = 128 * mi
    while s < B:
        w = min(512, B - s)
        out.append((s, w))
        s += w
    return out


N_SEG = sum(len(_segs(mi)) for mi in range(8))  # 12
DS_OFF = {}  # mi -> packed column offset of DS storage
_o = 0
for _mi in range(8):
    DS_OFF[_mi] = _o
    _o += B - 128 * _mi
DS_W = _o  # 4608


def _build():
    nc = bacc.Bacc("TRN2", target_bir_lowering=False, debug=False, num_devices=NCORES)

    xs_ap = nc.dram_tensor("xs", [SECPC, B, D], f32, kind="ExternalInput").ap()
    xsec_ap = nc.dram_tensor("xsec", [N, BSH, D], f32, kind="ExternalInput").ap()
    enc_ap = nc.dram_tensor("enc", [B, D], f32, kind="ExternalInput").ap()
    idb_ap = nc.dram_tensor("identb", [128, 128], fp16, kind="ExternalInput").ap()
    um_ap = nc.dram_tensor("umask", [32, 512], f32, kind="ExternalInput").ap()
    o_sent_ap = nc.dram_tensor("o_sent", [128, 2], f32, kind="ExternalOutput").ap()
    o_sec_ap = nc.dram_tensor("o_sec", [32, 1], f32, kind="ExternalOutput").ap()

    with tile.TileContext(nc) as tc:
        _body(tc, nc, xs_ap, xsec_ap, enc_ap, idb_ap, um_ap, o_sent_ap, o_sec_ap)
    nc.compile()
    return nc


def _body(tc, nc, xs_ap, xsec_ap, enc_ap, idb_ap, um_ap, o_sent_ap, o_sec_ap):
    import contextlib

    with contextlib.ExitStack() as ctx:
        cpool = ctx.enter_context(tc.tile_pool(name="consts", bufs=1))
        spool = ctx.enter_context(tc.tile_pool(name="slots", bufs=1))
        dram_pool = ctx.enter_context(tc.tile_pool(name="dram", bufs=1, space="DRAM"))

        ident_b = cpool.tile([128, 128], fp16, tag="identb")
        nc.sync.dma_start(ident_b[:], idb_ap[:])
        umask = cpool.tile([32, 512], f32, tag="umask")
        nc.sync.dma_start(umask[:], um_ap[:])
        eps_t = cpool.tile([128, 1], f32, tag="epst")
        nc.vector.memset(eps_t[:], EPS)
        ones128 = cpool.tile([1, 128], fp16, tag="ones128")
        nc.vector.memset(ones128[:], 1.0)
        ones32 = cpool.tile([1, 32], fp16, tag="ones32")
        nc.vector.memset(ones32[:], 1.0)

        sent_slots = spool.tile([128, SECPC * N_SEG], f32, tag="sent_slots")
        accd_slots = spool.tile([128, SECPC * 8], f32, tag="accd_slots")
        sec_slots = spool.tile([32, 8], f32, tag="sec_slots")

        # ---------------- secret (pairwise margin) phase ----------------
        with contextlib.ExitStack() as sctx:
            xsn_pool = sctx.enter_context(tc.tile_pool(name="xsn", bufs=2))
            xts_pool = sctx.enter_context(tc.tile_pool(name="xtsec", bufs=1))
            sqs_pool = sctx.enter_context(tc.tile_pool(name="sqsec", bufs=1))
            junk_pool = sctx.enter_context(tc.tile_pool(name="sjunk", bufs=2))
            ptp_pool = sctx.enter_context(
                tc.tile_pool(name="ptp_s", bufs=3, space="PSUM")
            )
            pmm_pool = sctx.enter_context(
                tc.tile_pool(name="pmm_s", bufs=2, space="PSUM")
            )
            work_pool = sctx.enter_context(tc.tile_pool(name="swork", bufs=3))

            # xtsec[d, k, i, b] = outputs[i, 128c + b, 128k + d]
            xtsec = xts_pool.tile([128, 8, N, BSH], fp16, tag="xtsec")
            sqsec2 = sqs_pool.tile([128, N], f32, tag="sqsec2")  # 0.5*|x|^2
            for g in range(4):
                xsn = xsn_pool.tile([128, 8, D], fp16, tag="xsn")
                nc.gpsimd.dma_start(
                    xsn[:], xsec_ap[8 * g : 8 * g + 8].rearrange("i b d -> b i d")
                )
                for ii in range(8):
                    i = 8 * g + ii
                    junk = junk_pool.tile([128, D], fp16, tag="sjunk")
                    nc.scalar.activation(
                        out=junk[:],
                        in_=xsn[:, ii, :],
                        func=Act.Square,
                        scale=RSQRT2,
                        accum_out=sqsec2[:, i : i + 1],
                    )
                    for k in range(8):
                        pst = ptp_pool.tile([128, 128], fp16, tag="pst")
                        nc.tensor.transpose(
                            pst[:], xsn[:, ii, 128 * k : 128 * (k + 1)], ident_b[:]
                        )
                        if (i * 8 + k) % 2 == 0:
                            nc.vector.tensor_copy(xtsec[:, k, i, :], pst[:])
                        else:
                            nc.scalar.copy(xtsec[:, k, i, :], pst[:])
            # -0.5*|x|^2 in row-form [1, b, i] on partition 0 (matmul operands
            # must start at partition 0/32/64) — bounce through DRAM scratch.
            sqsecn = sqs_pool.tile([128, N], f32, tag="sqsecn")
            nc.scalar.activation(out=sqsecn[:], in_=sqsec2[:], func=Act.Copy, scale=-1.0)
            scr_sec = dram_pool.tile([BSH, N], f32, tag="scr_sec")
            nc.sync.dma_start(scr_sec[:], sqsecn[:])
            sqsrow = sqs_pool.tile([1, BSH, N], fp16, tag="sqsrow")
            nc.gpsimd.dma_start(sqsrow[:], scr_sec[:][None])

            for g8 in range(8):  # 16 b's per group
                ps = pmm_pool.tile([32, 512], f32, tag="ps_sec")
                for bb in range(16):
                    b = 16 * g8 + bb
                    c0 = 32 * bb
                    for k in range(8):
                        op = xtsec[:, k, :, b]
                        nc.tensor.matmul(
                            ps[:, c0 : c0 + 32], op, op, start=(k == 0), stop=False
                        )
                    nc.tensor.matmul(
                        ps[:, c0 : c0 + 32],
                        sqsrow[0:1, b, :],
                        ones32[:],
                        start=False,
                        stop=False,
                    )
                    nc.tensor.matmul(
                        ps[:, c0 : c0 + 32],
                        ones32[:],
                        sqsrow[0:1, b, :],
                        start=False,
                        stop=True,
                    )
                # ps = g - (sq_i + sq_j)/2 = -d2/2
                m = work_pool.tile([32, 512], f32, tag="smin")
                nc.vector.tensor_scalar(
                    out=m[:], in0=ps[:], scalar1=0.0, scalar2=None, op0=Alu.min
                )
                dse = work_pool.tile([32, 512], f32, tag="sdse")
                nc.scalar.activation(
                    out=dse[:], in_=m[:], func=Act.Sqrt, scale=-2.0, bias=eps_t[0:32]
                )
                hin = work_pool.tile([32, 512], f32, tag="shin")
                nc.scalar.activation(
                    out=hin[:], in_=dse[:], func=Act.Relu, scale=-1.0, bias=float(MARGIN)
                )
                junk2 = work_pool.tile([32, 512], f32, tag="sjunk2")
                nc.vector.scalar_tensor_tensor(
                    out=junk2[:],
                    in0=hin[:],
                    scalar=0.0,
                    in1=umask[:],
                    op0=Alu.bypass,
                    op1=Alu.mult,
                    accum_out=sec_slots[:, g8 : g8 + 1],
                )

        # ---------------- sentence (distance consistency) phase ----------------
        with contextlib.ExitStack() as tctx:
            xnat_pool = tctx.enter_context(tc.tile_pool(name="xnat", bufs=2))
            xtb_pool = tctx.enter_context(tc.tile_pool(name="xtb", bufs=2))
            sq_pool = tctx.enter_context(tc.tile_pool(name="sqp", bufs=2))
            ds_pool = tctx.enter_context(tc.tile_pool(name="dsp", bufs=1))
            junk_pool = tctx.enter_context(tc.tile_pool(name="tjunk", bufs=2))
            ptp_pool = tctx.enter_context(
                tc.tile_pool(name="ptp_t", bufs=3, space="PSUM")
            )
            pmm_pool = tctx.enter_context(
                tc.tile_pool(name="pmm_t", bufs=3, space="PSUM")
            )
            work_pool = tctx.enter_context(tc.tile_pool(name="twork", bufs=3))

            ds = ds_pool.tile([128, DS_W], f32, tag="ds")

            def process_matrix(src3d, is_ds, si_base, di_base):
                """src3d: [p, t, d] AP view (f32 in DRAM). Computes grams over the
                upper-triangle tile region; writes DS if is_ds else accumulates
                (d - ds)^2 into sent_slots/accd_slots."""
                xnat = xnat_pool.tile([128, 8, D], fp16, tag="xnat")
                nc.gpsimd.dma_start(xnat[:], src3d)
                sq2 = sq_pool.tile([128, 8], f32, tag="sq2")
                for t in range(8):
                    junk = junk_pool.tile([128, D], fp16, tag="tjunk")
                    nc.scalar.activation(
                        out=junk[:],
                        in_=xnat[:, t, :],
                        func=Act.Square,
                        scale=RSQRT2,
                        accum_out=sq2[:, t : t + 1],
                    )
                # sqrow[0, t, p] = -0.5*|x_(128t+p)|^2 in row-form on partition 0
                # (rank-1 matmul operand) — bounce through DRAM scratch.
                sqn2 = sq_pool.tile([128, 8], f32, tag="sqn2")
                nc.scalar.activation(out=sqn2[:], in_=sq2[:], func=Act.Copy, scale=-1.0)
                scr = dram_pool.tile([8, 128], f32, tag="scr_sent")
                nc.sync.dma_start(scr[:].rearrange("t p -> p t"), sqn2[:])
                sqrow = sq_pool.tile([1, 8, 128], fp16, tag="sqrow")
                nc.gpsimd.dma_start(sqrow[:], scr[:][None])

                xtb = xtb_pool.tile([128, 8, B], fp16, tag="xtb")
                for k in range(8):
                    for t in range(8):
                        pst = ptp_pool.tile([128, 128], fp16, tag="pstt")
                        nc.tensor.transpose(
                            pst[:], xnat[:, t, 128 * k : 128 * (k + 1)], ident_b[:]
                        )
                        if (k * 8 + t) % 2 == 0:
                            nc.vector.tensor_copy(
                                xtb[:, k, 128 * t : 128 * (t + 1)], pst[:]
                            )
                        else:
                            nc.scalar.copy(xtb[:, k, 128 * t : 128 * (t + 1)], pst[:])

                si = si_base
                di = di_base
                for mi in range(8):
                    for (s, w) in _segs(mi):
                        ps = pmm_pool.tile([128, 512], f32, tag="ps_mm")
                        for k in range(8):
                            nc.tensor.matmul(
                                ps[:, :w],
                                xtb[:, k, 128 * mi : 128 * (mi + 1)],
                                xtb[:, k, s : s + w],
                                start=(k == 0),
                                stop=False,
                            )
                        # rank-1 updates: add -0.5*sq_b along free columns
                        tlo = s // 128
                        thi = (s + w - 1) // 128
                        for t in range(tlo, thi + 1):
                            a0 = max(s, 128 * t)
                            a1 = min(s + w, 128 * (t + 1))
                            nc.tensor.matmul(
                                ps[:, a0 - s : a1 - s],
                                ones128[:],
                                sqrow[0:1, t, a0 - 128 * t : a1 - 128 * t],
                                start=False,
                                stop=(t == thi),
                            )
                        # m = min(g - sq_b/2 - sq_a/2, 0) = -relu(d2)/2
                        m = work_pool.tile([128, 512], f32, tag="tmin")
                        nc.vector.tensor_scalar(
                            out=m[:, :w],
                            in0=ps[:, :w],
                            scalar1=sq2[:, mi : mi + 1],
                            scalar2=0.0,
                            op0=Alu.subtract,
                            op1=Alu.min,
                        )
                        off = DS_OFF[mi] + (s - 128 * mi)
                        if is_ds:
                            nc.scalar.activation(
                                out=ds[:, off : off + w],
                                in_=m[:, :w],
                                func=Act.Sqrt,
                                scale=-2.0,
                                bias=eps_t[:],
                            )
                        else:
                            d = work_pool.tile([128, 512], f32, tag="td")
                            nc.scalar.activation(
                                out=d[:, :w],
                                in_=m[:, :w],
                                func=Act.Sqrt,
                                scale=-2.0,
                                bias=eps_t[:],
                            )
                            diff = work_pool.tile([128, 512], f32, tag="tdiff")
                            nc.vector.scalar_tensor_tensor(
                                out=diff[:, :w],
                                in0=d[:, :w],
                                scalar=0.0,
                                in1=ds[:, off : off + w],
                                op0=Alu.bypass,
                                op1=Alu.subtract,
                            )
                            junk2 = work_pool.tile([128, 512], f32, tag="tjunk2")
                            nc.vector.scalar_tensor_tensor(
                                out=junk2[:, :w],
                                in0=diff[:, :w],
                                scalar=0.0,
                                in1=diff[:, :w],
                                op0=Alu.bypass,
                                op1=Alu.mult,
                                accum_out=sent_slots[:, si : si + 1],
                            )
                            si += 1
                            if s == 128 * mi:
                                junk3 = work_pool.tile([128, 128], f32, tag="tjunk3")
                                nc.vector.scalar_tensor_tensor(
                                    out=junk3[:],
                                    in0=diff[:, :128],
                                    scalar=0.0,
                                    in1=diff[:, :128],
                                    op0=Alu.bypass,
                                    op1=Alu.mult,
                                    accum_out=accd_slots[:, di : di + 1],
                                )
                                di += 1

            process_matrix(enc_ap.rearrange("(t p) d -> p t d", p=128), True, 0, 0)
            for i in range(SECPC):
                process_matrix(
                    xs_ap[i].rearrange("(t p) d -> p t d", p=128),
                    False,
                    i * N_SEG,
                    i * 8,
                )

        # ---------------- final reduction + output ----------------
        with tc.tile_pool(name="outp", bufs=1) as opool:
            o_sent = opool.tile([128, 2], f32, tag="o_sent_sb")
            nc.vector.tensor_reduce(
                out=o_sent[:, 0:1], in_=sent_slots[:], axis=AxX, op=Alu.add
            )
            nc.vector.tensor_reduce(
                out=o_sent[:, 1:2], in_=accd_slots[:], axis=AxX, op=Alu.add
            )
            nc.sync.dma_start(o_sent_ap[:], o_sent[:])
            o_sec = opool.tile([32, 1], f32, tag="o_sec_sb")
            nc.vector.tensor_reduce(
                out=o_sec[:], in_=sec_slots[:], axis=AxX, op=Alu.add
            )
            nc.sync.dma_start(o_sec_ap[:], o_sec[:])


_NC_CACHE = None


def _get_nc():
    global _NC_CACHE
    if _NC_CACHE is None:
        _NC_CACHE = _build()
    return _NC_CACHE


def _host_inputs():
    ident_b = np.eye(128, dtype=np.float16)
    um = np.tile(np.triu(np.ones((32, 32), np.float32), 1), (1, 16))
    return ident_b, um


def run_on_device(outputs, encode_sentences, trace=False, **kw):
    nc = _get_nc()
    ident_b, um = _host_inputs()
    in_maps = []
    for c in range(NCORES):
        in_maps.append(
            {
                "xs": np.ascontiguousarray(outputs[SECPC * c : SECPC * (c + 1)]),
                "xsec": np.ascontiguousarray(outputs[:, BSH * c : BSH * (c + 1), :]),
                "enc": np.ascontiguousarray(encode_sentences),
                "identb": ident_b,
                "umask": um,
            }
        )
    return run_bass_kernel_spmd(nc, in_maps, list(range(NCORES)), trace=trace, **kw)


def _finish(results):
    sent_region = 0.0
    diag = 0.0
    sec = 0.0
    for c in range(NCORES):
        r = results[c]
        sent_region += r["o_sent"][:, 0].sum(dtype=np.float64)
        diag += r["o_sent"][:, 1].sum(dtype=np.float64)
        sec += r["o_sec"].sum(dtype=np.float64)
    total_sent = 2.0 * sent_region - diag
    sentence_loss = total_sent / (N * B * B)
    secret_loss = (sec / B) / (N * (N - 1) / 2.0)
    loss = ALPHA * sentence_loss + (1.0 - ALPHA) * secret_loss
    return (
        np.float32(loss),
        np.float32(sentence_loss),
        np.float32(secret_loss),
    )


def kernel(outputs, encode_sentences):
    res = run_on_device(outputs, encode_sentences)
    return _finish(res.results)


# revision 12
# speedup vs baseline: 1.1783x; 1.1783x over previous
"""Trainium2 Bass kernel for the contrastive loss problem.

Sharding: core c handles sentence-loss for secrets [4c, 4c+4) (upper-triangle
tiles of the BxB distance matrices, x2-minus-diagonal trick) and secret-loss
for batch columns [128c, 128c+128). Per-core scalar partials are summed on the
host (equivalent to the all-reduce of the scalar losses).
"""

import sys

sys.path.insert(0, "/opt/trn_rl_repo")

import numpy as np
import ml_dtypes

import concourse.bacc as bacc
import concourse.tile as tile
from concourse import mybir
from concourse.bass_utils import run_bass_kernel_spmd

N, B, D = 32, 1024, 1024
NCORES = 8
SECPC = N // NCORES  # 4 secrets per core (sentence term)
BSH = B // NCORES  # 128 batch columns per core (secret term)
EPS = 1e-12
MARGIN = 1.0
ALPHA = 0.5
RSQRT2 = 0.7071067811865476  # Square(x * 1/sqrt(2)) == x^2 / 2

f32 = mybir.dt.float32
bf16 = mybir.dt.bfloat16
fp16 = mybir.dt.float16
Alu = mybir.AluOpType
Act = mybir.ActivationFunctionType
AxX = mybir.AxisListType.X


def _segs(mi):
    """Column segments (start, width<=512) covering [128*mi, 1024)."""
    out = []
    s = 128 * mi
    while s < B:
        w = min(512, B - s)
        out.append((s, w))
        s += w
    return out


N_SEG = sum(len(_segs(mi)) for mi in range(8))  # 12
DS_OFF = {}  # mi -> packed column offset of DS storage
_o = 0
for _mi in range(8):
    DS_OFF[_mi] = _o
    _o += B - 128 * _mi
DS_W = _o  # 4608


def _build():
    nc = bacc.Bacc("TRN2", target_bir_lowering=False, debug=False, num_devices=NCORES)

    xs_ap = nc.dram_tensor("xs", [SECPC, B, D], f32, kind="ExternalInput").ap()
    xsec_ap = nc.dram_tensor("xsec", [N, BSH, D], f32, kind="ExternalInput").ap()
    enc_ap = nc.dram_tensor("enc", [B, D], f32, kind="ExternalInput").ap()
    idb_ap = nc.dram_tensor("identb", [128, 128], fp16, kind="ExternalInput").ap()
    um_ap = nc.dram_tensor("umask", [32, 512], f32, kind="ExternalInput").ap()
    o_sent_ap = nc.dram_tensor("o_sent", [128, 2], f32, kind="ExternalOutput").ap()
    o_sec_ap = nc.dram_tensor("o_sec", [32, 1], f32, kind="ExternalOutput").ap()

    with tile.TileContext(nc) as tc:
        _body(tc, nc, xs_ap, xsec_ap, enc_ap, idb_ap, um_ap, o_sent_ap, o_sec_ap)
    nc.compile()
    return nc


def _body(tc, nc, xs_ap, xsec_ap, enc_ap, idb_ap, um_ap, o_sent_ap, o_sec_ap):
    import contextlib

    with contextlib.ExitStack() as ctx:
        cpool = ctx.enter_context(tc.tile_pool(name="consts", bufs=1))
        spool = ctx.enter_context(tc.tile_pool(name="slots", bufs=1))
        dram_pool = ctx.enter_context(tc.tile_pool(name="dram", bufs=1, space="DRAM"))

        ident_b = cpool.tile([128, 128], fp16, tag="identb")
        nc.sync.dma_start(ident_b[:], idb_ap[:])
        umask = cpool.tile([32, 512], f32, tag="umask")
        nc.sync.dma_start(umask[:], um_ap[:])
        eps_t = cpool.tile([128, 1], f32, tag="epst")
        nc.vector.memset(eps_t[:], EPS)
        ones128 = cpool.tile([1, 128], fp16, tag="ones128")
        nc.vector.memset(ones128[:], 1.0)
        ones32 = cpool.tile([1, 32], fp16, tag="ones32")
        nc.vector.memset(ones32[:], 1.0)

        sent_slots = spool.tile([128, SECPC * N_SEG], f32, tag="sent_slots")
        accd_slots = spool.tile([128, SECPC * 8], f32, tag="accd_slots")
        sec_slots = spool.tile([32, 8], f32, tag="sec_slots")

        # ---------------- sentence (distance consistency) phase ----------------
        with contextlib.ExitStack() as tctx:
            xnat_pool = tctx.enter_context(tc.tile_pool(name="xnat", bufs=2))
            xtb_pool = tctx.enter_context(tc.tile_pool(name="xtb", bufs=2))
            sq_pool = tctx.enter_context(tc.tile_pool(name="sqp", bufs=2))
            ds_pool = tctx.enter_context(tc.tile_pool(name="dsp", bufs=1))
            junk_pool = tctx.enter_context(tc.tile_pool(name="tjunk", bufs=2))
            ptp_pool = tctx.enter_context(
                tc.tile_pool(name="ptp_t", bufs=4, space="PSUM")
            )
            pmm_pool = tctx.enter_context(
                tc.tile_pool(name="pmm_t", bufs=4, space="PSUM")
            )
            work_pool = tctx.enter_context(tc.tile_pool(name="twork", bufs=3))

            ds = ds_pool.tile([128, DS_W], f32, tag="ds")

            def process_matrix(src3d, is_ds, si_base, di_base):
                """src3d: [p, t, d] AP view (f32 in DRAM). Computes grams over the
                upper-triangle tile region; writes DS if is_ds else accumulates
                (d - ds)^2 into sent_slots/accd_slots."""
                xnat = xnat_pool.tile([128, 8, D], fp16, tag="xnat")
                nc.gpsimd.dma_start(xnat[:], src3d)
                sq2 = sq_pool.tile([128, 8], f32, tag="sq2")
                for t in range(8):
                    junk = junk_pool.tile([128, D], fp16, tag="tjunk")
                    nc.scalar.activation(
                        out=junk[:],
                        in_=xnat[:, t, :],
                        func=Act.Square,
                        scale=RSQRT2,
                        accum_out=sq2[:, t : t + 1],
                    )
                # sqrow[0, t, p] = -0.5*|x_(128t+p)|^2 in row-form on partition 0
                # (rank-1 matmul operand) — bounce through DRAM scratch.
                sqn2 = sq_pool.tile([128, 8], f32, tag="sqn2")
                nc.scalar.activation(out=sqn2[:], in_=sq2[:], func=Act.Copy, scale=-1.0)
                scr = dram_pool.tile([8, 128], f32, tag="scr_sent")
                nc.sync.dma_start(scr[:].rearrange("t p -> p t"), sqn2[:])
                sqrow = sq_pool.tile([1, 8, 128], fp16, tag="sqrow")
                nc.gpsimd.dma_start(sqrow[:], scr[:][None])

                xtb = xtb_pool.tile([128, 8, B], fp16, tag="xtb")
                for k in range(8):
                    for t in range(8):
                        pst = ptp_pool.tile([128, 128], fp16, tag="pstt")
                        nc.tensor.transpose(
                            pst[:], xnat[:, t, 128 * k : 128 * (k + 1)], ident_b[:]
                        )
                        nc.vector.tensor_copy(
                            xtb[:, k, 128 * t : 128 * (t + 1)], pst[:]
                        )

                si = si_base
                di = di_base
                for mi in range(8):
                    for (s, w) in _segs(mi):
                        ps = pmm_pool.tile([128, 512], f32, tag="ps_mm")
                        for k in range(8):
                            nc.tensor.matmul(
                                ps[:, :w],
                                xtb[:, k, 128 * mi : 128 * (mi + 1)],
                                xtb[:, k, s : s + w],
                                start=(k == 0),
                                stop=False,
                            )
                        # rank-1 updates: add -0.5*sq_b along free columns
                        tlo = s // 128
                        thi = (s + w - 1) // 128
                        for t in range(tlo, thi + 1):
                            a0 = max(s, 128 * t)
                            a1 = min(s + w, 128 * (t + 1))
                            nc.tensor.matmul(
                                ps[:, a0 - s : a1 - s],
                                ones128[:],
                                sqrow[0:1, t, a0 - 128 * t : a1 - 128 * t],
                                start=False,
                                stop=(t == thi),
                            )
                        # m = min(g - sq_b/2 - sq_a/2, 0) = -relu(d2)/2
                        m = work_pool.tile([128, 512], f32, tag="tmin")
                        nc.vector.tensor_scalar(
                            out=m[:, :w],
                            in0=ps[:, :w],
                            scalar1=sq2[:, mi : mi + 1],
                            scalar2=0.0,
                            op0=Alu.subtract,
                            op1=Alu.min,
                        )
                        off = DS_OFF[mi] + (s - 128 * mi)
                        if is_ds:
                            nc.scalar.activation(
                                out=ds[:, off : off + w],
                                in_=m[:, :w],
                                func=Act.Sqrt,
                                scale=-2.0,
                                bias=eps_t[:],
                            )
                        else:
                            d = work_pool.tile([128, 512], f32, tag="td")
                            nc.scalar.activation(
                                out=d[:, :w],
                                in_=m[:, :w],
                                func=Act.Sqrt,
                                scale=-2.0,
                                bias=eps_t[:],
                            )
                            diff = work_pool.tile([128, 512], f32, tag="tdiff")
                            nc.vector.scalar_tensor_tensor(
                                out=diff[:, :w],
                                in0=d[:, :w],
                                scalar=0.0,
                                in1=ds[:, off : off + w],
                                op0=Alu.bypass,
                                op1=Alu.subtract,
                            )
                            junk2 = work_pool.tile([128, 512], f32, tag="tjunk2")
                            nc.vector.scalar_tensor_tensor(
                                out=junk2[:, :w],
                                in0=diff[:, :w],
                                scalar=0.0,
                                in1=diff[:, :w],
                                op0=Alu.bypass,
                                op1=Alu.mult,
                                accum_out=sent_slots[:, si : si + 1],
                            )
                            si += 1
                            if s == 128 * mi:
                                junk3 = work_pool.tile([128, 128], f32, tag="tjunk3")
                                nc.vector.scalar_tensor_tensor(
                                    out=junk3[:],
                                    in0=diff[:, :128],
                                    scalar=0.0,
                                    in1=diff[:, :128],
                                    op0=Alu.bypass,
                                    op1=Alu.mult,
                                    accum_out=accd_slots[:, di : di + 1],
                                )
                                di += 1

            process_matrix(enc_ap.rearrange("(t p) d -> p t d", p=128), True, 0, 0)
            for i in range(SECPC):
                process_matrix(
                    xs_ap[i].rearrange("(t p) d -> p t d", p=128),
                    False,
                    i * N_SEG,
                    i * 8,
                )

        # ---------------- secret (pairwise margin) phase ----------------
        with contextlib.ExitStack() as sctx:
            xsn_pool = sctx.enter_context(tc.tile_pool(name="xsn", bufs=2))
            xts_pool = sctx.enter_context(tc.tile_pool(name="xtsec", bufs=1))
            sqs_pool = sctx.enter_context(tc.tile_pool(name="sqsec", bufs=1))
            junk_pool = sctx.enter_context(tc.tile_pool(name="sjunk", bufs=2))
            ptp_pool = sctx.enter_context(
                tc.tile_pool(name="ptp_s", bufs=3, space="PSUM")
            )
            pmm_pool = sctx.enter_context(
                tc.tile_pool(name="pmm_s", bufs=2, space="PSUM")
            )
            work_pool = sctx.enter_context(tc.tile_pool(name="swork", bufs=3))

            # xtsec[d, k, i, b] = outputs[i, 128c + b, 128k + d]
            xtsec = xts_pool.tile([128, 8, N, BSH], fp16, tag="xtsec")
            sqsec2 = sqs_pool.tile([128, N], f32, tag="sqsec2")  # 0.5*|x|^2
            for g in range(4):
                xsn = xsn_pool.tile([128, 8, D], fp16, tag="xsn")
                nc.gpsimd.dma_start(
                    xsn[:], xsec_ap[8 * g : 8 * g + 8].rearrange("i b d -> b i d")
                )
                for ii in range(8):
                    i = 8 * g + ii
                    junk = junk_pool.tile([128, D], fp16, tag="sjunk")
                    nc.scalar.activation(
                        out=junk[:],
                        in_=xsn[:, ii, :],
                        func=Act.Square,
                        scale=RSQRT2,
                        accum_out=sqsec2[:, i : i + 1],
                    )
                    for k in range(8):
                        pst = ptp_pool.tile([128, 128], fp16, tag="pst")
                        nc.tensor.transpose(
                            pst[:], xsn[:, ii, 128 * k : 128 * (k + 1)], ident_b[:]
                        )
                        nc.vector.tensor_copy(xtsec[:, k, i, :], pst[:])
            # -0.5*|x|^2 in row-form [1, b, i] on partition 0 (matmul operands
            # must start at partition 0/32/64) — bounce through DRAM scratch.
            sqsecn = sqs_pool.tile([128, N], f32, tag="sqsecn")
            nc.scalar.activation(out=sqsecn[:], in_=sqsec2[:], func=Act.Copy, scale=-1.0)
            scr_sec = dram_pool.tile([BSH, N], f32, tag="scr_sec")
            nc.sync.dma_start(scr_sec[:], sqsecn[:])
            sqsrow = sqs_pool.tile([1, BSH, N], fp16, tag="sqsrow")
            nc.gpsimd.dma_start(sqsrow[:], scr_sec[:][None])

            for g8 in range(8):  # 16 b's per group
                ps = pmm_pool.tile([32, 512], f32, tag="ps_sec")
                for bb in range(16):
                    b = 16 * g8 + bb
                    c0 = 32 * bb
                    for k in range(8):
                        op = xtsec[:, k, :, b]
                        nc.tensor.matmul(
                            ps[:, c0 : c0 + 32], op, op, start=(k == 0), stop=False
                        )
                    nc.tensor.matmul(
                        ps[:, c0 : c0 + 32],
                        sqsrow[0:1, b, :],
                        ones32[:],
                        start=False,
                        stop=False,
                    )
                    nc.tensor.matmul(
                        ps[:, c0 : c0 + 32],
                        ones32[:],
                        sqsrow[0:1, b, :],
                        start=False,
                        stop=True,
                    )
                # ps = g - (sq_i + sq_j)/2 = -d2/2
                m = work_pool.tile([32, 512], f32, tag="smin")
                nc.vector.tensor_scalar(
                    out=m[:], in0=ps[:], scalar1=0.0, scalar2=None, op0=Alu.min
                )
                dse = work_pool.tile([32, 512], f32, tag="sdse")
                nc.scalar.activation(
                    out=dse[:], in_=m[:], func=Act.Sqrt, scale=-2.0, bias=eps_t[0:32]
                )
                hin = work_pool.tile([32, 512], f32, tag="shin")
                nc.scalar.activation(
                    out=hin[:], in_=dse[:], func=Act.Relu, scale=-1.0, bias=float(MARGIN)
                )
                junk2 = work_pool.tile([32, 512], f32, tag="sjunk2")
                nc.vector.scalar_tensor_tensor(
                    out=junk2[:],
                    in0=hin[:],
                    scalar=0.0,
                    in1=umask[:],
                    op0=Alu.bypass,
                    op1=Alu.mult,
                    accum_out=sec_slots[:, g8 : g8 + 1],
                )

        # ---------------- final reduction + output ----------------
        with tc.tile_pool(name="outp", bufs=1) as opool:
            o_sent = opool.tile([128, 2], f32, tag="o_sent_sb")
            nc.vector.tensor_reduce(
                out=o_sent[:, 0:1], in_=sent_slots[:], axis=AxX, op=Alu.add
            )
            nc.vector.tensor_reduce(
                out=o_sent[:, 1:2], in_=accd_slots[:], axis=AxX, op=Alu.add
            )
            nc.sync.dma_start(o_sent_ap[:], o_sent[:])
            o_sec = opool.tile([32, 1], f32, tag="o_sec_sb")
            nc.vector.tensor_reduce(
                out=o_sec[:], in_=sec_slots[:], axis=AxX, op=Alu.add
            )
            nc.sync.dma_start(o_sec_ap[:], o_sec[:])


_NC_CACHE = None


def _get_nc():
    global _NC_CACHE
    if _NC_CACHE is None:
        _NC_CACHE = _build()
    return _NC_CACHE


def _host_inputs():
    ident_b = np.eye(128, dtype=np.float16)
    um = np.tile(np.triu(np.ones((32, 32), np.float32), 1), (1, 16))
    return ident_b, um


def run_on_device(outputs, encode_sentences, trace=False, **kw):
    nc = _get_nc()
    ident_b, um = _host_inputs()
    in_maps = []
    for c in range(NCORES):
        in_maps.append(
            {
                "xs": np.ascontiguousarray(outputs[SECPC * c : SECPC * (c + 1)]),
                "xsec": np.ascontiguousarray(outputs[:, BSH * c : BSH * (c + 1), :]),
                "enc": np.ascontiguousarray(encode_sentences),
                "identb": ident_b,
                "umask": um,
            }
        )
    return run_bass_kernel_spmd(nc, in_maps, list(range(NCORES)), trace=trace, **kw)


def _finish(results):
    sent_region = 0.0
    diag = 0.0
    sec = 0.0
    for c in range(NCORES):
        r = results[c]
        sent_region += r["o_sent"][:, 0].sum(dtype=np.float64)
        diag += r["o_sent"][:, 1].sum(dtype=np.float64)
        sec += r["o_sec"].sum(dtype=np.float64)
    total_sent = 2.0 * sent_region - diag
    sentence_loss = total_sent / (N * B * B)
    secret_loss = (sec / B) / (N * (N - 1) / 2.0)
    loss = ALPHA * sentence_loss + (1.0 - ALPHA) * secret_loss
    return (
        np.float32(loss),
        np.float32(sentence_loss),
        np.float32(secret_loss),
    )


def kernel(outputs, encode_sentences):
    res = run_on_device(outputs, encode_sentences)
    return _finish(res.results)
